# revision 31
# baseline (speedup 1.0000x reference)
# Bass/Trainium2 kernel for nn_M2R_25778393710941 (loss_fn).
#
# reference:
#   proj_j = Mj @ W.T ; proj_i = Mi @ W.T            [B, K]
#   pos = einsum('bk,bk->b', proj_j, r[:, rp].T)
#   neg = einsum('bk,bk->b', proj_i, r[:, ri].T)
#   loss = relu(pos - neg + 1).mean()
#
# Shapes: B=4096, NV=16384, NR=10000, K=128.
#
# Strategy (8 cores, data-parallel over batch; BS=512 rows per core):
#   - Host: cast M shards to fp8e4m3 and pack as [p, k, b] (k = 128-row
#     contraction block) so every DMA reads long contiguous per-partition runs;
#     pack W (scaled by K, lossless) to WT[p, k*128+m] = K*W[m, k*128+p] so
#     matmul operands load in natural contraction-on-partition layout; gather
#     r columns (r[:, rp] is already [K, B] layout).
#   - Device: projT[kw, b] += WT_blk.T @ MT_blk accumulated over the 128
#     nv-blocks into PSUM via fp8 DoubleRow matmuls (256 contraction rows per
#     pass; redundant Ldweights deduped), two banks: pos from Mj, neg from Mi.
#     Then d = projT_pos*rpT - projT_neg*riT (DVE), column-sum over the
#     partition dim via +/-ones matmuls, scale by 1/K, DMA out the per-sample
#     margins. Host applies +1/relu/mean.
import os
import sys

import numpy as np
import ml_dtypes

B, NV, NR, K = 4096, 16384, 10000, 128
NCORES = 8
BS = B // NCORES          # 512 batch rows per core
P = 128                   # partition dim / nv-block size
NBLK = NV // P            # 128 contraction blocks
# nv-blocks per SBUF buffer chunk: small leading chunks prime the pipeline
# fast, big middle chunks amortize, small tail chunks cut the final PE burst.
CHUNKS = [4, 4, 8, 16, 32, 32, 24, 8]
assert sum(CHUNKS) == NBLK
M_DT = "float8e4"         # dtype of the streamed M operand (matmul rhs)
W_DT = "float8e4"         # dtype of the resident W operand (matmul lhsT)

# W is pre-scaled by K (=128, a power of two, lossless) on the host so its
# entries have ~unit variance — required for fp8 W. The epilogue multiplies
# the reduced margins by 1/K to undo it.
_NP_DT = {
    "bfloat16": np.dtype(ml_dtypes.bfloat16),
    "float8e4": np.dtype(ml_dtypes.float8_e4m3),
    "float32": np.dtype(np.float32),
}

_NC = None                # cached compiled Bass program
LAST_RESULTS = None       # stashed BassKernelResults for test.py introspection


def _build_bass():
    import concourse.bacc as bacc
    import concourse.mybir as mybir
    import concourse.tile as tile
    from concourse.bass import ts

    mdt = getattr(mybir.dt, M_DT)
    wdt = getattr(mybir.dt, W_DT)
    f32 = mybir.dt.float32
    bf16 = mybir.dt.bfloat16

    nc = bacc.Bacc(
        "TRN2",
        target_bir_lowering=False,
        debug=False,
        enable_asserts=False,
        num_devices=NCORES,
    )

    # M shards host-packed to [p, k, b] so chunk DMAs read long contiguous
    # per-partition runs (ch*BS bytes) instead of strided 512 B segments.
    mjt_d = nc.dram_tensor("mjt", [P, NBLK, BS], mdt, kind="ExternalInput")
    mit_d = nc.dram_tensor("mit", [P, NBLK, BS], mdt, kind="ExternalInput")
    wt_d = nc.dram_tensor("wt", [P, NV], wdt, kind="ExternalInput")
    rpt_d = nc.dram_tensor("rpt", [P, BS], f32, kind="ExternalInput")
    rit_d = nc.dram_tensor("rit", [P, BS], f32, kind="ExternalInput")
    losses_d = nc.dram_tensor("losses", [1, BS], f32, kind="ExternalOutput")
    ones_d = nc.inline_tensor(
        np.ones((P, 1), ml_dtypes.bfloat16), name="ones_c"
    )
    nones_d = nc.inline_tensor(
        np.full((P, 1), -1.0, ml_dtypes.bfloat16), name="nones_c"
    )

    with tile.TileContext(nc) as tc:
        with (
            tc.tile_pool(name="wt", bufs=1) as wt_pool,
            tc.tile_pool(name="m", bufs=5) as m_pool,
            tc.tile_pool(name="consts", bufs=1) as c_pool,
            tc.tile_pool(name="ep", bufs=1) as ep_pool,
            tc.tile_pool(name="ps", bufs=1, space="PSUM") as ps_pool,
        ):
            # Resident packed W.T: the slice the first chunk needs rides the
            # fast Sync queue; the rest prefetches on the GpSimd queue in the
            # background, off the hot M streams.
            wt_sb = wt_pool.tile([P, NV], wdt)
            nc.sync.dma_start(
                out=wt_sb[:, : CHUNKS[0] * P], in_=wt_d[:, : CHUNKS[0] * P]
            )
            nc.gpsimd.dma_start(
                out=wt_sb[:, CHUNKS[0] * P :], in_=wt_d[:, CHUNKS[0] * P :]
            )

            rpt_sb = c_pool.tile([P, BS], f32, tag="rpt")
            nc.gpsimd.dma_start(out=rpt_sb[:], in_=rpt_d[:])
            rit_sb = c_pool.tile([P, BS], f32, tag="rit")
            nc.gpsimd.dma_start(out=rit_sb[:], in_=rit_d[:])
            ones_sb = c_pool.tile([P, 1], bf16, tag="ones")
            nc.gpsimd.dma_start(out=ones_sb[:], in_=ones_d[:])
            nones_sb = c_pool.tile([P, 1], bf16, tag="nones")
            nc.gpsimd.dma_start(out=nones_sb[:], in_=nones_d[:])

            ps_pos = ps_pool.tile([P, BS], f32, tag="pos")
            ps_neg = ps_pool.tile([P, BS], f32, tag="neg")

            # Scratch operands for HAM-warmth filler matmuls (see loop below).
            wsc_sb = c_pool.tile([P, 1], mdt, tag="wsc")
            nc.vector.memset(wsc_sb[:], 1.0)
            xsc_sb = c_pool.tile([P, P], mdt, tag="xsc")
            nc.vector.memset(xsc_sb[:], 0.125)
            ps_warm = ps_pool.tile([1, P], f32, tag="warm")

            blk0 = 0
            for c, ch in enumerate(CHUNKS):
                # Split each chunk's transfer into <=8-block DMAs so matmuls
                # can start on the first sub-slice while the rest streams in
                # (Tile tracks sub-tile ranges), keeping PE idle gaps short.
                mj_sb = m_pool.tile([P, ch, BS], mdt, tag="mj")
                mi_sb = m_pool.tile([P, ch, BS], mdt, tag="mi")
                for s0 in range(0, ch, 8):
                    w = min(8, ch - s0)
                    nc.sync.dma_start(
                        out=mj_sb[:, s0 : s0 + w, :],
                        in_=mjt_d[:, blk0 + s0 : blk0 + s0 + w, :],
                    )
                    nc.scalar.dma_start(
                        out=mi_sb[:, s0 : s0 + w, :],
                        in_=mit_d[:, blk0 + s0 : blk0 + s0 + w, :],
                    )
                # DoubleRow: one matmul consumes two contraction blocks —
                # lhsT [K, 2, M], rhs [K, 2, N] -> out += W0.T@X0 + W1.T@X1.
                for k in range(0, ch, 2):
                    kk = blk0 + k
                    wpair = wt_sb[:, kk * P : (kk + 2) * P].rearrange(
                        "p (two m) -> p two m", two=2
                    )
                    nc.tensor.matmul(
                        ps_pos[:],
                        wpair,
                        mj_sb[:, k : k + 2, :],
                        start=(kk == 0),
                        stop=(kk == NBLK - 2),
                        perf_mode=mybir.MatmulPerfMode.DoubleRow,
                    )
                    nc.tensor.matmul(
                        ps_neg[:],
                        wpair,
                        mi_sb[:, k : k + 2, :],
                        start=(kk == 0),
                        stop=(kk == NBLK - 2),
                        perf_mode=mybir.MatmulPerfMode.DoubleRow,
                    )
                # Filler matmuls on scratch data: no data deps, so the
                # scheduler hoists them to the front of the PE stream where
                # they bridge the framework preamble and first-chunk DMA,
                # keeping the PE activity monitor from throttling the clock
                # to 1.2 GHz before the real matmul stream gets going.
                if 2 <= c < len(CHUNKS) - 1:
                    for _ in range(16):
                        nc.tensor.matmul(
                            ps_warm[:], wsc_sb[:], xsc_sb[:],
                            start=True, stop=True,
                        )
                blk0 += ch

            # d = ps_pos * rpT - ps_neg * riT, then column-sum over partitions.
            t_sb = ep_pool.tile([P, BS], bf16, tag="t")
            nc.vector.tensor_tensor(
                out=t_sb[:], in0=ps_pos[:], in1=rpt_sb[:], op=mybir.AluOpType.mult
            )
            u_sb = ep_pool.tile([P, BS], bf16, tag="u")
            nc.vector.tensor_tensor(
                out=u_sb[:], in0=ps_neg[:], in1=rit_sb[:], op=mybir.AluOpType.mult
            )
            ps_d = ps_pool.tile([1, BS], f32, tag="d")
            nc.tensor.matmul(ps_d[:], ones_sb[:], t_sb[:], start=True, stop=False)
            nc.tensor.matmul(ps_d[:], nones_sb[:], u_sb[:], start=False, stop=True)

            # Output pre-relu margins d/K; the (+1, relu, mean) tail runs on
            # the host. Avoids the ScalarE activation + its bias-constant
            # table load in the device epilogue.
            losses_sb = ep_pool.tile([1, BS], f32, tag="losses")
            nc.vector.tensor_scalar_mul(losses_sb[:], ps_d[:], 1.0 / K)
            nc.sync.dma_start(out=losses_d[:], in_=losses_sb[:])

    _dedup_ldweights(nc, mybir)
    nc.compile()
    return nc


def _dedup_ldweights(nc, mybir):
    """Tile lowering emits a standalone Ldweights before every Matmult, even
    when consecutive matmuls share the same stationary operand (our pos/neg
    pair). The PE keeps weights loaded across matmuls, so drop a Ldweights
    that exactly repeats the previous one (only Matmults in between, no sync
    attached). Halves PE weight-load traffic."""
    removed = 0
    for blk in nc.m.functions[0].blocks:
        insts = blk.instructions
        last_key = None
        to_remove = []
        for inst in insts:
            if inst.opcode == "Ldweights":
                key = (str(inst.ins), str(getattr(inst, "perf_mode", None)))
                si = inst.sync_info
                has_sync = si is not None and (
                    list(si.on_wait) or list(si.on_update)
                )
                if key == last_key and not has_sync:
                    to_remove.append(inst)
                else:
                    last_key = key
            elif inst.opcode == "Matmult":
                pass  # stationary weights survive matmuls
            elif inst.engine == mybir.EngineType.PE:
                last_key = None
        for inst in to_remove:
            insts.remove(inst)
        removed += len(to_remove)


def _get_nc():
    global _NC
    if _NC is None:
        _NC = _build_bass()
    return _NC


def _prep_inputs(Mi, Mj, ri, rp, W, r):
    Mi = np.asarray(Mi, dtype=np.float32)
    Mj = np.asarray(Mj, dtype=np.float32)
    ri = np.asarray(ri)
    rp = np.asarray(rp)
    W = np.asarray(W, dtype=np.float32)
    r = np.asarray(r, dtype=np.float32)

    mdt = _NP_DT[M_DT]
    wdt = _NP_DT[W_DT]

    # WT[p, k*P + m] = K * W[m, k*P + p] (contraction block k natural on
    # partitions; the K pre-scale is undone by the epilogue activation scale).
    wt = np.ascontiguousarray(
        (W * np.float32(K)).reshape(K, NBLK, P).transpose(2, 1, 0).reshape(P, NV)
    ).astype(wdt)

    rpt = r[:, rp]  # [K, B]
    rit = r[:, ri]  # [K, B]

    in_maps = []
    for s in range(NCORES):
        sl = slice(s * BS, (s + 1) * BS)
        def pack(M):
            # [BS, NV] -> [NV, BS] cast -> [p, k, b] contiguous
            t = M[sl].T.astype(mdt, order="C")
            return np.ascontiguousarray(
                t.reshape(NBLK, P, BS).transpose(1, 0, 2)
            )

        in_maps.append(
            {
                "mjt": pack(Mj),
                "mit": pack(Mi),
                "wt": wt,
                "rpt": np.ascontiguousarray(rpt[:, sl]),
                "rit": np.ascontiguousarray(rit[:, sl]),
            }
        )
    return in_maps


def kernel(Mi, Mj, ri, rp, W, r):
    from concourse.bass_utils import run_bass_kernel_spmd

    global LAST_RESULTS
    nc = _get_nc()
    in_maps = _prep_inputs(Mi, Mj, ri, rp, W, r)
    # NTFF tracing needs the antenv.axon_hooks shim (test.py installs it);
    # without it the axon trace path raises, so force tracing off.
    trace = bool(os.environ.get("BASS_TRACE"))
    if "antenv.axon_hooks" not in sys.modules:
        trace = False
        os.environ["BASS_NEVER_TRACE"] = "1"
    res = run_bass_kernel_spmd(
        nc, in_maps, core_ids=list(range(NCORES)), trace=trace
    )
    LAST_RESULTS = res
    margins = np.concatenate([out["losses"][0] for out in res.results])
    losses = np.maximum(margins.astype(np.float64) + 1.0, 0.0)
    return np.float32(np.mean(losses))
